# revision 6
# baseline (speedup 1.0000x reference)
"""Causal self-attention on 8 TRN2 NeuronCores.

Problem: x[4, 2048, 1024], qkv_w[1024, 3072], o_w[1024, 1024] (f32).
Sharding: core c = (batch b = c // 2, head-group g = c % 2 of 8 heads).
Each core computes qkv projection for its (batch, 8 heads), causal
attention, and a partial o_proj ([2048, 1024], f32).  Host sums the two
head-group partials per batch (the "all-reduce") and adds o_b.

Device-side layout choices:
  - All matmuls in bf16 (f32 PSUM accumulate); host pre-casts inputs.
  - Host passes x transposed (xT [1024, 2048]) so the d-contraction
    operands are already partition-major.
  - Q^T/K^T are produced in [channels, t] layout directly (lhsT = W).
  - Scores are computed transposed, S^T[k, q] = (K Q^T)/..., so the
    softmax k-sum can ride the PE: V is augmented with a ones column
    and Y^T_aug = [V|1]^T @ P^T gives the rowsum in row 64.
  - exp has no max-subtraction (scores are ~N(0,1); safe in f32).
  - Causality: per k-tile only columns q >= 128*floor(i/4)*... i.e. the
    live q range is computed per tile; the 128-wide diagonal triangle is
    masked with a precomputed upper-triangular bf16 mask.
  - Normalization (divide by rowsum) happens after the PV matmul on
    [64, 512] tiles via a DMA partition-broadcast reciprocal.
"""

from contextlib import ExitStack

import numpy as np
import ml_dtypes

import concourse.bass as bass
import concourse.tile as tile
from concourse import bacc, mybir
from concourse.bass_utils import run_bass_kernel_spmd
from concourse.masks import make_upper_triangular

BF16 = mybir.dt.bfloat16
F32 = mybir.dt.float32
AF = mybir.ActivationFunctionType

T = 2048          # sequence length
D = 1024          # model dim
HD = 64           # head dim
H_LOC = 8         # heads per core
DH = H_LOC * HD   # 512: local qkv width per core
NT = T // 128     # 16 t-tiles
NKD = D // 128    # 8 d k-tiles
NKH = DH // 128   # 4 hd k-tiles
SCALE = 1.0 / np.sqrt(np.float32(HD))  # 0.125


def _make_pools(ctx: ExitStack, tc: tile.TileContext):
    return {
        "persist": ctx.enter_context(tc.tile_pool(name="persist", bufs=1)),
        "ptiles": ctx.enter_context(tc.tile_pool(name="ptiles", bufs=16)),
        "recip": ctx.enter_context(tc.tile_pool(name="recip", bufs=4)),
        "recipb": ctx.enter_context(tc.tile_pool(name="recipb", bufs=4)),
        "outsb": ctx.enter_context(tc.tile_pool(name="outsb", bufs=4)),
        "recipd": ctx.enter_context(tc.tile_pool(name="recipd", bufs=4, space="DRAM")),
        "mmps": ctx.enter_context(tc.tile_pool(name="mmps", bufs=2, space="PSUM")),
        "sps": ctx.enter_context(tc.tile_pool(name="sps", bufs=4, space="PSUM")),
        "ytps": ctx.enter_context(tc.tile_pool(name="ytps", bufs=2, space="PSUM")),
    }


def _build_body(pools: dict, tc: tile.TileContext, io: dict):
    nc = tc.nc
    xt, wq, wk, wv, wo = io["xt"], io["wq"], io["wk"], io["wv"], io["wo"]
    qb, kb, vb, out = io["qb"], io["kb"], io["vb"], io["out"]

    persist = pools["persist"]
    p_pool = pools["ptiles"]
    rc_pool = pools["recip"]
    rb_pool = pools["recipb"]
    ob_pool = pools["outsb"]
    rd_pool = pools["recipd"]
    mm_ps = pools["mmps"]
    s_ps = pools["sps"]
    yt_ps = pools["ytps"]

    # ---- persistent SBUF tensors + loads -------------------------------
    xt_sb = persist.tile([128, NKD, T], BF16)
    nc.sync.dma_start(out=xt_sb, in_=xt.ap().rearrange("(i p) t -> p i t", p=128))
    wq_sb = persist.tile([128, NKD, DH], BF16)
    nc.sync.dma_start(out=wq_sb, in_=wq.ap().rearrange("(i p) n -> p i n", p=128))
    wk_sb = persist.tile([128, NKD, DH], BF16)
    nc.sync.dma_start(out=wk_sb, in_=wk.ap().rearrange("(i p) n -> p i n", p=128))
    wv_sb = persist.tile([128, NKD, DH], BF16)
    nc.sync.dma_start(out=wv_sb, in_=wv.ap().rearrange("(i p) n -> p i n", p=128))
    wo_sb = persist.tile([128, NKH, D], BF16)
    nc.sync.dma_start(out=wo_sb, in_=wo.ap().rearrange("(i p) n -> p i n", p=128))

    qb_sb = persist.tile([128, 4], F32)
    nc.sync.dma_start(out=qb_sb, in_=qb.ap().rearrange("(r p) -> p r", p=128))
    kb_sb = persist.tile([128, 4], F32)
    nc.sync.dma_start(out=kb_sb, in_=kb.ap().rearrange("(r p) -> p r", p=128))
    vb_sb = persist.tile([128, DH], F32)
    vb_ap = vb.ap()
    vb_bcast = bass.AP(tensor=vb_ap.tensor, offset=vb_ap.offset,
                       ap=[[0, 128]] + list(vb_ap.ap))
    nc.gpsimd.dma_start(out=vb_sb, in_=vb_bcast)

    tri = persist.tile([128, 128], BF16)
    make_upper_triangular(nc, tri[:], val=1.0, diag=True)

    # V with a ones column per (t-tile, head): [128, t-tile, head, 65]
    v_aug = persist.tile([128, NT, H_LOC, HD + 1], BF16)
    nc.vector.memset(v_aug[:], 1.0)

    qT_sb = persist.tile([128, 4, T], BF16)   # Q^T: [p, r, t], ch = 128r + p
    kT_sb = persist.tile([128, 4, T], BF16)
    yT_sb = persist.tile([128, NKH, T], BF16)  # Y^T (normalized attention out)

    # ---- V projection (natural layout, per t-tile) ---------------------
    def emit_v_tile(m):
        ps = mm_ps.tile([128, 512], F32, tag="mmps")
        for i in range(NKD):
            nc.tensor.matmul(ps, lhsT=xt_sb[:, i, 128 * m:128 * (m + 1)],
                             rhs=wv_sb[:, i, :],
                             start=(i == 0), stop=(i == NKD - 1))
        nc.vector.tensor_add(
            out=v_aug[:, m, :, 0:HD],
            in0=ps.rearrange("p (h e) -> p h e", e=HD),
            in1=vb_sb.rearrange("p (h e) -> p h e", e=HD),
        )

    # ---- Q^T / K^T projection for one 128-channel row tile r -----------
    def emit_qkT_row(w_sb, b_sb, dst, r):
        for c in range(4):
            ps = mm_ps.tile([128, 512], F32, tag="mmps")
            for i in range(NKD):
                nc.tensor.matmul(ps, lhsT=w_sb[:, i, 128 * r:128 * (r + 1)],
                                 rhs=xt_sb[:, i, 512 * c:512 * (c + 1)],
                                 start=(i == 0), stop=(i == NKD - 1))
            nc.scalar.activation(out=dst[:, r, 512 * c:512 * (c + 1)], in_=ps,
                                 func=AF.Identity, bias=b_sb[:, r:r + 1], scale=1.0)

    # ---- attention for one head pair (2*hp, 2*hp+1) --------------------
    def emit_attention_pair(hp):
        heads = [(2 * hp, 0), (2 * hp + 1, 64)]  # (local head, partition base)
        q_of = {h: qT_sb[pb:pb + 64, hp, :] for h, pb in heads}
        k_of = {h: kT_sb[pb:pb + 64, hp, :] for h, pb in heads}
        for j in range(4):  # q chunks of 512
            if hp == 0:  # V tiles just in time for the first pair
                for m in range(4 * j, 4 * j + 4):
                    emit_v_tile(m)
            n_k = 4 * j + 4
            p_tiles = {}
            for i in range(n_k):
                s = max(512 * j, 128 * i)
                w = 512 * j + 512 - s
                for h, pb in heads:
                    ps = s_ps.tile([128, 512], F32, tag="sps")
                    nc.tensor.matmul(ps[:, 0:w],
                                     lhsT=k_of[h][:, 128 * i:128 * (i + 1)],
                                     rhs=q_of[h][:, s:s + w],
                                     start=True, stop=True)
                    pt = p_pool.tile([128, 512], BF16, tag="pt")
                    nc.scalar.activation(out=pt[:, 0:w], in_=ps[:, 0:w],
                                         func=AF.Exp, scale=float(SCALE))
                    if i >= 4 * j:  # diagonal tile: mask the leading triangle
                        nc.vector.tensor_mul(pt[:, 0:128], pt[:, 0:128], tri)
                    p_tiles[(h, i)] = (pt, s, w)
            for h, pb in heads:
                yt = yt_ps.tile([65, 512], F32, tag="ytps")
                for i in range(n_k):
                    pt, s, w = p_tiles[(h, i)]
                    off = s - 512 * j
                    nc.tensor.matmul(yt[:, off:off + w],
                                     lhsT=v_aug[:, i, h, :],
                                     rhs=pt[:, 0:w],
                                     start=(i == 0), stop=(i == n_k - 1))
                rc = rc_pool.tile([1, 512], F32, tag="rc")
                nc.vector.reciprocal(rc, yt[64:65, :])
                # partition-broadcast via DRAM roundtrip (SBUF source APs
                # cannot have a zero partition step; DRAM sources can)
                rd = rd_pool.tile([512], F32, tag="rd")
                nc.sync.dma_start(out=rd, in_=rc)
                rb = rb_pool.tile([64, 512], F32, tag="rb")
                rd_ap = rd[:]
                rd_bcast = bass.AP(tensor=rd_ap.tensor, offset=rd_ap.offset,
                                   ap=[[0, 64]] + list(rd_ap.ap))
                nc.sync.dma_start(out=rb, in_=rd_bcast)
                nc.vector.tensor_mul(
                    out=yT_sb[pb:pb + 64, hp, 512 * j:512 * (j + 1)],
                    in0=yt[0:64, :], in1=rb)

    # ---- emission order ------------------------------------------------
    for hp in range(4):
        emit_qkT_row(wq_sb, qb_sb, qT_sb, hp)
        emit_qkT_row(wk_sb, kb_sb, kT_sb, hp)
        emit_attention_pair(hp)

    # ---- o_proj partial: out = Y^T.T @ Wo ------------------------------
    for m in range(NT):
        for c in range(2):
            ps = mm_ps.tile([128, 512], F32, tag="mmps")
            for kt in range(NKH):
                nc.tensor.matmul(ps, lhsT=yT_sb[:, kt, 128 * m:128 * (m + 1)],
                                 rhs=wo_sb[:, kt, 512 * c:512 * (c + 1)],
                                 start=(kt == 0), stop=(kt == NKH - 1))
            ob = ob_pool.tile([128, 512], F32, tag="ob")
            nc.scalar.activation(out=ob, in_=ps, func=AF.Copy)
            nc.sync.dma_start(out=out.ap()[128 * m:128 * (m + 1),
                                           512 * c:512 * (c + 1)], in_=ob)


def build_nc(loop_reps: int = 1):
    nc = bacc.Bacc("TRN2", target_bir_lowering=False, debug=False, num_devices=8)
    io = {
        "xt": nc.dram_tensor("xt", [D, T], BF16, kind="ExternalInput"),
        "wq": nc.dram_tensor("wq", [D, DH], BF16, kind="ExternalInput"),
        "wk": nc.dram_tensor("wk", [D, DH], BF16, kind="ExternalInput"),
        "wv": nc.dram_tensor("wv", [D, DH], BF16, kind="ExternalInput"),
        "wo": nc.dram_tensor("wo", [DH, D], BF16, kind="ExternalInput"),
        "qb": nc.dram_tensor("qb", [DH], F32, kind="ExternalInput"),
        "kb": nc.dram_tensor("kb", [DH], F32, kind="ExternalInput"),
        "vb": nc.dram_tensor("vb", [DH], F32, kind="ExternalInput"),
        "out": nc.dram_tensor("out", [T, D], F32, kind="ExternalOutput"),
    }
    with tile.TileContext(nc) as tc:
        with ExitStack() as ctx:
            pools = _make_pools(ctx, tc)
            if loop_reps > 1:  # benchmarking build: repeat the body in-NEFF
                with tc.For_i(0, loop_reps, 1):
                    _build_body(pools, tc, io)
            else:
                _build_body(pools, tc, io)
    nc.compile()
    return nc


def make_in_maps(x, qkv_w, qkv_b):
    bf = ml_dtypes.bfloat16
    x = np.asarray(x, np.float32)
    qkv_w = np.asarray(qkv_w, np.float32)
    qkv_b = np.asarray(qkv_b, np.float32)
    in_maps = []
    for c in range(8):
        b, g = divmod(c, 2)
        sl = slice(DH * g, DH * (g + 1))
        in_maps.append({
            "xt": np.ascontiguousarray(x[b].T).astype(bf),
            "wq": np.ascontiguousarray(qkv_w[:, DH * g:DH * (g + 1)]).astype(bf),
            "wk": np.ascontiguousarray(qkv_w[:, D + DH * g:D + DH * (g + 1)]).astype(bf),
            "wv": np.ascontiguousarray(qkv_w[:, 2 * D + DH * g:2 * D + DH * (g + 1)]).astype(bf),
            "wo": None,  # filled below (needs o_w)
            "qb": np.ascontiguousarray(qkv_b[sl]).astype(np.float32),
            "kb": np.ascontiguousarray(qkv_b[D + DH * g:D + DH * (g + 1)]).astype(np.float32),
            "vb": np.ascontiguousarray(qkv_b[2 * D + DH * g:2 * D + DH * (g + 1)]).astype(np.float32),
        })
    return in_maps


_NC_CACHE = {}


def get_nc():
    if "nc" not in _NC_CACHE:
        _NC_CACHE["nc"] = build_nc()
    return _NC_CACHE["nc"]


def kernel(x, qkv_w, qkv_b, o_w, o_b):
    x = np.asarray(x, np.float32)
    o_w = np.asarray(o_w, np.float32)
    o_b = np.asarray(o_b, np.float32)
    bf = ml_dtypes.bfloat16

    in_maps = make_in_maps(x, qkv_w, qkv_b)
    for c in range(8):
        g = c % 2
        in_maps[c]["wo"] = np.ascontiguousarray(o_w[DH * g:DH * (g + 1), :]).astype(bf)

    nc = get_nc()
    res = run_bass_kernel_spmd(nc, in_maps, core_ids=list(range(8))).results

    out = np.empty((4, T, D), np.float32)
    for b in range(4):
        out[b] = res[2 * b]["out"] + res[2 * b + 1]["out"]
    out += o_b[None, None, :]
    return out
